# revision 7
# baseline (speedup 1.0000x reference)
"""BSMatchStar Trainium2 kernel (v2).

out = (a | (((a&b) +_brev b) ^ b))  -- bitstream MatchStar via a 2^29-bit
big-integer addition over per-byte bit-reversed operands.

v2 layout: per core, the 8 MiB byte-slice is viewed as uint32 words
[128 partitions, 16384 words] with each partition owning a CONTIGUOUS
16384-word chunk of the stream. Tiles slice the free dim ([128, F]); the
carry scan chains across tiles through a [128,1] running state, so there
is no per-tile cross-row stitch at all. The only cross-partition carry
(row boundaries, core boundaries) is resolved on the host by walking the
all-propagate prefix of each row (expected O(1) bytes/row): the kernel
runs every row with carry-in 0 and exports the per-row aggregate state
(0=kill / 1=generate / 2=all-propagate) via the seed-2 scan trick.

Per tile:
  tr = brev(a&b), br = brev(b)  (SWAR: 3 levels x (2 shift-mask TS + OR))
  s  = tr + br                  (gpsimd: exact uint32 add)
  p8 = (~s == 0), g01 = msb(tr | (br & ~s))   -- int8 limb flags
  lc = tensor_tensor_scan(p8, g01, state; mult, max)  state chains tiles
  ci = lc_exclusive & 1 ; s' = s + ci (gpsimd)
  out = brev(s' ^ br) | a
"""
import sys
sys.path.insert(0, "/opt/trn_rl_repo")

import numpy as np

N_BYTES = 67_108_864
N_CORES = 8
P = 128
F = 2048
WORDS_PER_CORE = N_BYTES // 4 // N_CORES   # 2,097,152
ROW_WORDS = WORDS_PER_CORE // P            # 16,384
T = ROW_WORDS // F                         # 8
ROW_BYTES = ROW_WORDS * 4                  # 65,536

_BREV = np.array([int(f"{i:08b}"[::-1], 2) for i in range(256)], dtype=np.uint8)

_cache = {}

ML = [0xF0F0F0F0, 0xCCCCCCCC, 0xAAAAAAAA]
MR = [0x0F0F0F0F, 0x33333333, 0x55555555]
SH = [4, 2, 1]


def _build(n_tiles, f):
    import concourse.bacc as bacc
    import concourse.tile as tile
    import concourse.mybir as mybir
    import contextlib

    AOT = mybir.AluOpType
    dt = mybir.dt

    nc = bacc.Bacc("TRN2", target_bir_lowering=False, debug=False)

    nf = n_tiles * f
    d_a = nc.dram_tensor("a", [P, nf], dt.uint32, kind="ExternalInput")
    d_b = nc.dram_tensor("b", [P, nf], dt.uint32, kind="ExternalInput")
    d_o = nc.dram_tensor("o", [P, nf], dt.uint32, kind="ExternalOutput")
    d_rs = nc.dram_tensor("rowst", [P, 1], dt.int8, kind="ExternalOutput")

    with tile.TileContext(nc) as tc, contextlib.ExitStack() as ctx:
        pool = ctx.enter_context(tc.tile_pool(name="sb", bufs=1))
        iop = ctx.enter_context(tc.tile_pool(name="io", bufs=2))
        smp = ctx.enter_context(tc.tile_pool(name="sm", bufs=2))

        state = smp.tile([P, 1], dt.float32, tag="state")
        nc.vector.memset(state[:], 2.0)

        def brev(dst_tag, src, ts, comb):
            """per-byte bit reversal; comb[i] = engine for level-i OR."""
            x = src
            for li in range(3):
                A = pool.tile([P, f], dt.uint32, tag=f"{ts}A", name=f"{ts}A")
                B = pool.tile([P, f], dt.uint32, tag=f"{ts}B", name=f"{ts}B")
                nc.vector.tensor_scalar(A[:], x[:], SH[li], ML[li],
                                        AOT.logical_shift_left,
                                        AOT.bitwise_and)
                nc.vector.tensor_scalar(B[:], x[:], SH[li], MR[li],
                                        AOT.logical_shift_right,
                                        AOT.bitwise_and)
                y = pool.tile([P, f], dt.uint32,
                              tag=(dst_tag if li == 2 else f"{ts}Y"),
                              name=f"{ts}y{li}")
                eng = comb[li]
                op = AOT.add if eng is nc.gpsimd else AOT.bitwise_or
                eng.tensor_tensor(y[:], A[:], B[:], op)
                x = y
            return x

        for t in range(n_tiles):
            sl = slice(t * f, (t + 1) * f)
            a_t = iop.tile([P, f], dt.uint32, tag="a_t")
            b_t = iop.tile([P, f], dt.uint32, tag="b_t")
            nc.sync.dma_start(a_t[:], d_a[:, sl])
            nc.sync.dma_start(b_t[:], d_b[:, sl])

            t0 = pool.tile([P, f], dt.uint32, tag="t0")
            nc.vector.tensor_tensor(t0[:], a_t[:], b_t[:], AOT.bitwise_and)

            # disjoint-mask combines: integer ADD == OR, and gpsimd only
            # supports int add (not 32-bit bitwise), so gpsimd combines
            # use AOT.add
            tr = brev("tr", t0, "v", [nc.gpsimd, nc.vector, nc.vector])
            br = brev("br", b_t, "u", [nc.gpsimd, nc.vector, nc.vector])

            s = pool.tile([P, f], dt.uint32, tag="s")
            nc.gpsimd.tensor_tensor(s[:], tr[:], br[:], AOT.add)

            nots = pool.tile([P, f], dt.uint32, tag="nots")
            nc.vector.tensor_scalar(nots[:], s[:], 0xFFFFFFFF, None,
                                    AOT.bitwise_xor)
            p8 = pool.tile([P, f], dt.int8, tag="p8")
            nc.vector.tensor_scalar(p8[:], nots[:], 0, None, AOT.is_equal)
            n1 = pool.tile([P, f], dt.uint32, tag="n1")
            nc.vector.tensor_tensor(n1[:], nots[:], br[:], AOT.bitwise_and)
            gg = pool.tile([P, f], dt.uint32, tag="t0", name="gg")
            nc.vector.tensor_tensor(gg[:], n1[:], tr[:], AOT.bitwise_or)
            # bitVec ops can't cast dtypes; compare ops can. msb -> {0,1} int8
            gmsb = pool.tile([P, f], dt.uint32, tag="n1", name="gmsb")
            nc.vector.tensor_scalar(gmsb[:], gg[:], 0x80000000, None,
                                    AOT.bitwise_and)
            g01 = pool.tile([P, f], dt.int8, tag="g01")
            nc.vector.tensor_scalar(g01[:], gmsb[:], 2147483648.0, None,
                                    AOT.is_equal)

            # inclusive scan into cols 1..f; col 0 = incoming state, so
            # lcb[:, 0:f] is the exclusive carry-in per limb
            lcb = pool.tile([P, f + 1], dt.int8, tag="lcb")
            nc.vector.tensor_copy(lcb[:, 0:1], state[:])
            nc.vector.tensor_tensor_scan(lcb[:, 1:f + 1], p8[:], g01[:],
                                         state[:], AOT.mult, AOT.max)
            nstate = smp.tile([P, 1], dt.float32, tag="state", name="nstate")
            nc.vector.tensor_copy(nstate[:], lcb[:, f:f + 1])
            state = nstate

            # carry-in per limb: state values {0,1,2} -> {0,1,0}; is_equal
            # casts int8 -> uint32 (bitVec ops can't)
            ci = pool.tile([P, f], dt.uint32, tag="ci")
            nc.vector.tensor_scalar(ci[:], lcb[:, 0:f], 1, None,
                                    AOT.is_equal)

            sp = pool.tile([P, f], dt.uint32, tag="nots", name="sp")
            nc.gpsimd.tensor_tensor(sp[:], s[:], ci[:], AOT.add)
            w = pool.tile([P, f], dt.uint32, tag="n1", name="w")
            nc.vector.tensor_tensor(w[:], sp[:], br[:], AOT.bitwise_xor)

            wb = brev("wb", w, "v", [nc.vector, nc.vector, nc.vector])
            o_t = iop.tile([P, f], dt.uint32, tag="o_t")
            nc.vector.tensor_tensor(o_t[:], wb[:], a_t[:], AOT.bitwise_or)
            nc.sync.dma_start(d_o[:, sl], o_t[:])

        rs8 = smp.tile([P, 1], dt.int8, tag="rs8")
        nc.vector.tensor_copy(rs8[:], state[:])
        nc.sync.dma_start(d_rs[:], rs8[:])

    nc.compile()
    return nc


def _get_nc(n_tiles, f):
    key = (n_tiles, f)
    if key not in _cache:
        _cache[key] = _build(n_tiles, f)
    return _cache[key]


def run_sharded(a_u8, b_u8, n_cores=N_CORES, f=F, **spmd_kwargs):
    """Run the SPMD kernel over n_cores contiguous shards. Returns
    (out_u8_without_boundary_fixup, list[row_states int8[128]])."""
    from concourse import bass_utils

    n = a_u8.size
    words = n // 4
    wpc = words // n_cores
    n_tiles = wpc // (P * f)
    assert n_tiles * P * f == wpc, (n, n_cores, f)

    a32 = a_u8.view(np.uint32).reshape(n_cores, P, n_tiles * f)
    b32 = b_u8.view(np.uint32).reshape(n_cores, P, n_tiles * f)

    nc = _get_nc(n_tiles, f)
    in_maps = [{"a": np.ascontiguousarray(a32[c]),
                "b": np.ascontiguousarray(b32[c])}
               for c in range(n_cores)]
    res = bass_utils.run_bass_kernel_spmd(nc, in_maps,
                                          core_ids=list(range(n_cores)),
                                          **spmd_kwargs)
    outs = [r["o"] for r in res.results]
    rowstates = [r["rowst"].reshape(-1).astype(np.int8) for r in res.results]
    out = np.concatenate([o.reshape(-1).view(np.uint8) for o in outs])
    return out, rowstates, res


def _fixup_boundaries(out, a_u8, b_u8, rowstates, n_cores):
    """Resolve row-boundary carries on the host (decoupled lookback).

    Each row (core c, partition p) of ROW_BYTES bytes was computed with
    carry-in 0. Walk rows in stream order; when the true carry-in is 1,
    patch the row's all-propagate prefix (out = a|b) and bump the first
    non-propagate byte. Expected O(1) bytes of work per row.
    """
    carry = 0
    for c in range(n_cores):
        st = rowstates[c]
        for p in range(P):
            if carry:
                base = c * P * ROW_BYTES + p * ROW_BYTES
                i = base
                en = base + ROW_BYTES
                done = False
                while i < en and not done:
                    j = min(i + 4096, en)
                    aa = a_u8[i:j]
                    bb = b_u8[i:j]
                    raw = (_BREV[aa & bb].astype(np.int32)
                           + _BREV[bb].astype(np.int32))
                    prop = raw == 255
                    if prop.all():
                        out[i:j] = aa | bb
                        i = j
                        continue
                    k = int(np.argmin(prop))  # first non-propagate byte
                    out[i:i + k] = aa[:k] | bb[:k]
                    idx = i + k
                    new_s = (int(raw[k]) + 1) & 0xFF
                    out[idx] = ((int(_BREV[new_s]) ^ int(b_u8[idx]))
                                | int(a_u8[idx]))
                    done = True
            sv = int(st[p])
            carry = 1 if sv == 1 else (carry if sv == 2 else 0)
    return out


def kernel(a, b):
    assert a.dtype == np.uint8 and b.dtype == np.uint8 and a.size == N_BYTES
    out, rowstates, _ = run_sharded(a, b)
    out = _fixup_boundaries(out, a, b, rowstates, N_CORES)
    return out
